# revision 13
# baseline (speedup 1.0000x reference)
"""Causal self-attention on 8 TRN2 NeuronCores.

Reference computation (B=4, T=2048, C=1024, H=16, D=64, fp32):
    qkv = x @ W_attn + b_attn ; split q,k,v ; per-head causal softmax(q k^T / 8) @ v
    y = heads @ W_proj + b_proj

Sharding: core c handles batch b = c//2 and head-half hh = c%2 (8 heads).
QKV weights are column-split and W_proj row-split per core, so each core
computes an independent partial projection; the host sums the two partials
per batch and adds the bias terms (b_proj and the folded-out v-bias
contribution b_v @ W_proj, which is constant because softmax rows sum to 1).
No collectives, no redundant FLOPs.

Per-core kernel layout:
  - qT, kT are feature-major [head*64, T]; v is token-major with a ones
    column per head ([T, 8 x (64 v | 1)]).
  - S^T tiles [k=128, q<=512] come from K=64 matmuls, two heads sharing the
    PE array via partition-offset row groups; exp reads both heads' PSUM
    banks in one ACT op; causal masking is a DVE multiply with a triangle
    mask on the diagonal 128-blocks only; above-diagonal work is skipped at
    128-col granularity (suffix trimming) for every q-quarter.
  - A@V runs "flipped": the exp tile is the stationary operand and the
    augmented v chunk [64 v | 1] the moving one, so each matmul's output is
    [128 q, 65] instead of [65, 512] - ~2x less PE streaming for the same
    math. Row 64 accumulates the softmax denominator. One DVE divide per
    (head, q-subtile) normalizes straight out of PSUM into a bf16 [q, d]
    tile, and a single DMA-transpose per (head-pair, quarter) rebuilds the
    feature-major yT used as the projection's stationary operand.
  - Emission order doubles as scheduler priority: QKV groups of the next
    t-quarter and projection groups of finished quarters are interleaved
    between attention head-pairs, so the PE always has ready work while
    the ACT engine grinds through exp.

Dtypes: matmul inputs in bf16 (halves DMA; fp32 PSUM accumulation keeps the
end-to-end error ~2.6e-3 against the 2e-2 gate).
"""

import numpy as np
import ml_dtypes

import concourse.bacc as bacc
import concourse.mybir as mybir
import concourse.tile as tile
from concourse.bass_utils import run_bass_kernel_spmd

F32 = mybir.dt.float32
BF16 = mybir.dt.bfloat16
FP8 = mybir.dt.float8e4
AF = mybir.ActivationFunctionType
ALU = mybir.AluOpType
DR = mybir.MatmulPerfMode.DoubleRow

N_CORES = 8
B, T, C = 4, 2048, 1024
H, D = 16, 64
CH = 512            # features per core (8 heads * 64)
NFO = 4             # head-pair chunks of 128 features
NTQ = 4             # t quarters of 512
NTC = 16            # t chunks of 128
SCALE = 0.125       # 1/sqrt(64)

_cached = {}


def _build_nc():
    nc = bacc.Bacc("TRN2", debug=False, num_devices=N_CORES)

    d_xT = nc.dram_tensor("xT", [C, T], BF16, kind="ExternalInput")
    d_wq = nc.dram_tensor("wq", [C, CH], BF16, kind="ExternalInput")
    d_wk = nc.dram_tensor("wk", [C, CH], BF16, kind="ExternalInput")
    d_wv = nc.dram_tensor("wv", [C, CH], BF16, kind="ExternalInput")
    d_bq = nc.dram_tensor("bq", [128, NFO], F32, kind="ExternalInput")
    d_bk = nc.dram_tensor("bk", [128, NFO], F32, kind="ExternalInput")
    d_wp = nc.dram_tensor("wp", [CH, C], BF16, kind="ExternalInput")
    d_masks = nc.dram_tensor("masks", [128, 128], BF16, kind="ExternalInput")
    d_out = nc.dram_tensor("out", [T, C], F32, kind="ExternalOutput")

    with tile.TileContext(nc) as tc, nc.allow_low_precision(
        reason="bf16 staging; accumulation stays fp32 in PSUM"
    ), (
        tc.tile_pool(name="persist", bufs=1)
    ) as persist, (
        tc.tile_pool(name="pW", bufs=1)
    ) as pW, (
        tc.tile_pool(name="pX", bufs=1)
    ) as pX, (
        tc.tile_pool(name="pO", bufs=4)
    ) as pO, (
        tc.tile_pool(name="p2e", bufs=6)
    ) as p2e, (
        tc.tile_pool(name="pY", bufs=2)
    ) as pY, (
        tc.tile_pool(name="psA", bufs=2, space="PSUM")
    ) as psA, (
        tc.tile_pool(name="psS", bufs=2, space="PSUM")
    ) as psS, (
        tc.tile_pool(name="psY", bufs=1, space="PSUM")
    ) as psY:
        # persistent on-chip tensors.
        # q/k live in fp8 DoubleRow layout: tile g in {0,1} holds head-pairs
        # fo = 2g, 2g+1; partition p = (fo%2)*64 + j*32 + dk and free slot
        # (i, t) hold feature d = 32*i + dk of head 2*fo+j. A DoubleRow
        # matmul contracts the (partition, slot) pair - 64 d-values on 32
        # partitions - at half the PE cost per output column.
        qdr = [persist.tile([128, 2, T], FP8, tag=f"qdr{g}", name=f"qdr{g}") for g in range(2)]
        kdr = [persist.tile([128, 2, T], FP8, tag=f"kdr{g}", name=f"kdr{g}") for g in range(2)]
        v = [persist.tile([128, 8, 65], BF16, tag=f"v{i}", name=f"v{i}") for i in range(NTC)]
        yT = [persist.tile([128, T], BF16, tag=f"yT{fo}", name=f"yT{fo}") for fo in range(NFO)]
        bq_sb = persist.tile([128, NFO], F32, tag="bq")
        bk_sb = persist.tile([128, NFO], F32, tag="bk")
        masks_sb = persist.tile([128, 128], BF16, tag="masks")
        wq_sb = pW.tile([128, 8, CH], BF16, tag="wq")
        wk_sb = pW.tile([128, 8, CH], BF16, tag="wk")
        wv_sb = pW.tile([128, 8, CH], BF16, tag="wv")
        wp_sb = pW.tile([128, 4, C], BF16, tag="wp")
        x_tiles = [pX.tile([128, 8, 512], BF16, tag=f"x{tq}", name=f"x{tq}") for tq in range(NTQ)]

        # input DMAs: first-needed tensors split chunk-size so the first
        # QKV matmuls can start as soon as possible; later tensors ride in
        # bigger transfers (per-DMA dispatch is ~650ns, keep the count low)
        def _x_piece(tq, c0, c1, t0=0, t1=512):
            nc.sync.dma_start(
                x_tiles[tq][:, c0:c1, t0:t1],
                d_xT.ap()[128 * c0 : 128 * c1, 512 * tq + t0 : 512 * tq + t1].rearrange(
                    "(c p) t -> p c t", p=128
                ),
            )

        def _w_piece(dst, src, c0, c1):
            nc.sync.dma_start(
                dst[:, c0:c1, :],
                src.ap()[128 * c0 : 128 * c1, :].rearrange("(c p) f -> p c f", p=128),
            )

        _x_piece(0, 0, 1)
        _w_piece(wq_sb, d_wq, 0, 1)
        nc.sync.dma_start(bq_sb[:], d_bq.ap())
        nc.sync.dma_start(bk_sb[:], d_bk.ap())
        _x_piece(0, 1, 2)
        _w_piece(wq_sb, d_wq, 1, 2)
        _x_piece(0, 2, 4)
        _w_piece(wq_sb, d_wq, 2, 4)
        _x_piece(0, 4, 8)
        _w_piece(wq_sb, d_wq, 4, 8)
        _w_piece(wk_sb, d_wk, 0, 4)
        _w_piece(wv_sb, d_wv, 0, 4)
        _w_piece(wk_sb, d_wk, 4, 8)
        _w_piece(wv_sb, d_wv, 4, 8)
        nc.sync.dma_start(masks_sb[:], d_masks.ap())
        for tq in range(1, NTQ):
            nc.sync.dma_start(
                x_tiles[tq][:],
                d_xT.ap()[:, 512 * tq : 512 * (tq + 1)].rearrange("(c p) t -> p c t", p=128),
            )
        nc.sync.dma_start(wp_sb[:], d_wp.ap().rearrange("(c p) f -> p c f", p=128))

        def emit_qk_group(bq_, w_sb, b_sb, dst, fo):
            # two M=64 accumulation chains per head-pair (one per DoubleRow
            # slot i), partition-disjoint halves of one PSUM bank; the host
            # pre-permutes W columns to [fo, i, j, dk] order so each chain's
            # output partitions line up with the fp8 DoubleRow tile layout.
            g, pbase = fo // 2, (fo % 2) * 64
            ps = psA.tile([128, 512], F32, tag="psA", name="ps_qk")
            for i in range(2):
                for ci in range(8):
                    nc.tensor.matmul(
                        ps[64 * i : 64 * (i + 1), :],
                        w_sb[:, ci, 128 * fo + 64 * i : 128 * fo + 64 * (i + 1)],
                        x_tiles[bq_][:, ci, :],
                        start=(ci == 0),
                        stop=(ci == 7),
                    )
            for i in range(2):
                nc.vector.tensor_scalar(
                    dst[g][pbase : pbase + 64, i, 512 * bq_ : 512 * (bq_ + 1)],
                    ps[64 * i : 64 * (i + 1), :],
                    b_sb[pbase : pbase + 64, 2 * (fo // 2) + i : 2 * (fo // 2) + i + 1],
                    None,
                    op0=ALU.add,
                )

        def emit_v_group(bq_, ts_):
            tci = 4 * bq_ + ts_
            ps = psA.tile([128, 512], F32, tag="psA", name="ps_v")
            for ci in range(8):
                nc.tensor.matmul(
                    ps[:],
                    x_tiles[bq_][:, ci, 128 * ts_ : 128 * (ts_ + 1)],
                    wv_sb[:, ci, :],
                    start=(ci == 0),
                    stop=(ci == 7),
                )
            nc.vector.memset(v[tci][:, :, 64:65], 1.0)
            nc.vector.tensor_copy(
                v[tci][:, :, 0:64],
                ps[:].rearrange("p (h d) -> p h d", h=8),
            )

        def emit_qkv_group(bq_, g):
            if g < 4:
                emit_qk_group(bq_, wq_sb, bq_sb, qdr, g)
            elif g < 8:
                emit_qk_group(bq_, wk_sb, bk_sb, kdr, g - 4)
            else:
                emit_v_group(bq_, g - 8)

        def emit_proj_tc(tci):
            o_sb = pO.tile([128, C], F32, tag="o", name="o_sb")
            for co in range(2):
                ps = psA.tile([128, 512], F32, tag="psA", name="ps_o")
                for fo in range(NFO):
                    nc.tensor.matmul(
                        ps[:],
                        yT[fo][:, 128 * tci : 128 * (tci + 1)],
                        wp_sb[:, fo, 512 * co : 512 * (co + 1)],
                        start=(fo == 0),
                        stop=(fo == 3),
                    )
                # both copies on DVE: ACT must stay free for exp, which is
                # the serial resource gating the attention pipeline
                nc.vector.tensor_copy(o_sb[:, 512 * co : 512 * (co + 1)], ps[:])
                nc.sync.dma_start(
                    d_out.ap()[128 * tci : 128 * (tci + 1), 512 * co : 512 * (co + 1)],
                    o_sb[:, 512 * co : 512 * (co + 1)],
                )

        def emit_attn(fo, b):
            hA, hB = 2 * fo, 2 * fo + 1
            q0 = 512 * b
            # A@V accumulators: per head, 4 q-subtiles of [128 q, 65] at
            # 128-col offsets inside one PSUM bank; col 64 is the softmax
            # denominator fed by the ones column of v.
            pyA = psY.tile([128, 512], F32, tag="pyA")
            pyB = psY.tile([128, 512], F32, tag="pyB")
            # zero the accumulator regions explicitly: matmul start=True
            # would lazily zero the whole 2KB zero-region (the bank), which
            # breaks interleaved per-subtile accumulation groups. The memset
            # overlaps every region, so it also orders all A@V matmuls after
            # it regardless of scheduler priority.
            nc.vector.memset(pyA[:].rearrange("p (s c) -> p s c", c=128)[:, :, 0:65], 0.0)
            nc.vector.memset(pyB[:].rearrange("p (s c) -> p s c", c=128)[:, :, 0:65], 0.0)
            # diagonal chunks first (kc = 4b+i), then full rows; chunk i only
            # reaches q-subtiles s >= i, everything above the diagonal is
            # skipped at 128-col granularity.
            order = [4 * b + i for i in range(4)] + list(range(4 * b))
            n_row = 4 * b  # full-row chunks
            for idx, kc in enumerate(order):
                i = kc - 4 * b  # >=0 for diagonal chunks
                qoff = 0 if i <= 0 else 128 * i
                pS = psS.tile([128, 1024], F32, tag="pS", name="pS")
                g = fo // 2
                for j, ooff in ((0, 0), (1, 512)):
                    pb = (fo % 2) * 64 + 32 * j
                    nc.tensor.matmul(
                        pS[:, ooff + qoff : ooff + 512],
                        kdr[g][pb : pb + 32, :, 128 * kc : 128 * (kc + 1)],
                        qdr[g][pb : pb + 32, :, q0 + qoff : q0 + 512],
                        perf_mode=DR,
                        tile_position=(pb, 0),
                    )
                eST = p2e.tile([128, 1024], BF16, tag="eST", name="eST")
                if qoff == 0:
                    nc.scalar.activation(eST[:], pS[:], AF.Exp, scale=SCALE)
                else:
                    # one ACT op over both heads' valid suffixes, skipping the
                    # [512, 512+qoff) hole via a strided AP
                    pS2 = pS[:].rearrange("p (two n) -> p two n", two=2)
                    eST2 = eST[:].rearrange("p (two n) -> p two n", two=2)
                    nc.scalar.activation(
                        eST2[:, :, qoff:512], pS2[:, :, qoff:512], AF.Exp, scale=SCALE
                    )
                if i >= 0:
                    # causal triangle mask on the diagonal 128-block
                    for off in (qoff, 512 + qoff):
                        nc.vector.tensor_tensor(
                            eST[:, off : off + 128],
                            eST[:, off : off + 128],
                            masks_sb[:],
                            op=ALU.mult,
                        )
                subs = range(i, 4) if i >= 0 else range(4)
                for s in subs:
                    nc.tensor.matmul(
                        pyA[:, 128 * s : 128 * s + 65],
                        eST[:, 128 * s : 128 * (s + 1)],
                        v[kc][:, hA, :],
                        start=False,
                        stop=False,
                        skip_group_check=True,
                    )
                    nc.tensor.matmul(
                        pyB[:, 128 * s : 128 * s + 65],
                        eST[:, 512 + 128 * s : 512 + 128 * (s + 1)],
                        v[kc][:, hB, :],
                        start=False,
                        stop=False,
                        skip_group_check=True,
                    )
            # normalize: reciprocal of the per-q denominators (col 64 of each
            # region), then one DVE multiply per (head, q-subtile) straight
            # out of PSUM into the bf16 [q, (s, d)] staging tile; a single
            # DMA-transpose rebuilds feature-major yT.
            yQ = pY.tile([128, 4, 128], BF16, tag="yQ", name="yQ")
            recA = pY.tile([128, 4], F32, tag="recA", name="recA")
            recB = pY.tile([128, 4], F32, tag="recB", name="recB")
            pyA4 = pyA[:].rearrange("p (s c) -> p s c", c=128)
            pyB4 = pyB[:].rearrange("p (s c) -> p s c", c=128)
            nc.vector.reciprocal(recA[:], pyA4[:, :, 64:65])
            nc.vector.reciprocal(recB[:], pyB4[:, :, 64:65])
            for s in range(4):
                nc.vector.tensor_scalar(
                    yQ[:, s, 0:64],
                    pyA[:, 128 * s : 128 * s + 64],
                    recA[:, s : s + 1],
                    None,
                    op0=ALU.mult,
                )
                nc.vector.tensor_scalar(
                    yQ[:, s, 64:128],
                    pyB[:, 128 * s : 128 * s + 64],
                    recB[:, s : s + 1],
                    None,
                    op0=ALU.mult,
                )
            nc.sync.dma_start_transpose(
                yT[fo][:, q0 : q0 + 512].rearrange("p (s q) -> p s q", s=4),
                yQ[:],
            )

        # Emission order doubles as scheduler priority: between attention
        # head-pairs (whose inner loop is ACT-bound) we emit PE-dense filler:
        # the NEXT quarter's QKV groups, or projection groups of a finished
        # quarter, so the PE always has ready work.
        for g in range(12):
            emit_qkv_group(0, g)
        for b in range(NTQ):
            for fo in range(NFO):
                emit_attn(fo, b)
                if b < NTQ - 1:
                    for g in range(3 * fo, 3 * fo + 3):
                        emit_qkv_group(b + 1, g)
                if b == 2:
                    emit_proj_tc(fo)          # proj quarter 0
                elif b == 3:
                    # b=3 has no QKV filler left; give each head-pair
                    # boundary two projection units (quarters 1 and 2)
                    emit_proj_tc(4 + fo)
                    emit_proj_tc(8 + fo)
        for tci in range(12, 16):
            emit_proj_tc(tci)

    nc.compile()
    return nc


def _get_nc():
    if "nc" not in _cached:
        _cached["nc"] = _build_nc()
    return _cached["nc"]


def kernel(x, W_attn, b_attn, W_proj, b_proj):
    x = np.asarray(x, np.float32)
    W_attn = np.asarray(W_attn, np.float32)
    b_attn = np.asarray(b_attn, np.float32)
    W_proj = np.asarray(W_proj, np.float32)
    b_proj = np.asarray(b_proj, np.float32)

    nc = _get_nc()
    p = np.arange(128)[:, None]
    j = np.arange(128)[None, :]
    tri = (j >= p).astype(np.float32)          # [128,128] valid iff j >= p
    masks = tri.astype(ml_dtypes.bfloat16)     # [128, 128]
    masks_u16 = masks.view(np.uint16)

    # q/k weight-column permutation for the fp8 DoubleRow layout: new column
    # fo*128 + i*64 + j*32 + dk holds feature (2*fo+j)*64 + 32*i + dk, so the
    # two M=64 QKV chains land directly on the DoubleRow partition layout.
    fo_, i_, j_, dk_ = np.meshgrid(
        np.arange(4), np.arange(2), np.arange(2), np.arange(32), indexing="ij"
    )
    perm = ((2 * fo_ + j_) * 64 + 32 * i_ + dk_).reshape(-1)   # [512]

    def dr_bias(bvec):
        # [128, 4]: partition (fo%2)*64 + j*32 + dk, column (fo//2)*2 + i
        arr = np.zeros((128, 4), np.float32)
        arr[((fo_ % 2) * 64 + j_ * 32 + dk_).reshape(-1),
            ((fo_ // 2) * 2 + i_).reshape(-1)] = bvec[perm]
        return arr

    in_maps = []
    for c in range(N_CORES):
        b, hh = divmod(c, 2)
        sl = slice(CH * hh, CH * (hh + 1))
        in_maps.append(
            {
                "xT": np.ascontiguousarray(x[b].T).astype(ml_dtypes.bfloat16).view(np.uint16),
                "wq": np.ascontiguousarray(W_attn[:, 0:C][:, sl][:, perm]).astype(ml_dtypes.bfloat16).view(np.uint16),
                "wk": np.ascontiguousarray(W_attn[:, C : 2 * C][:, sl][:, perm]).astype(ml_dtypes.bfloat16).view(np.uint16),
                "wv": np.ascontiguousarray(W_attn[:, 2 * C : 3 * C][:, sl]).astype(ml_dtypes.bfloat16).view(np.uint16),
                "bq": dr_bias(b_attn[0:C][sl]),
                "bk": dr_bias(b_attn[C : 2 * C][sl]),
                "wp": np.ascontiguousarray(
                    W_proj[sl, :].astype(ml_dtypes.bfloat16)
                ).view(np.uint16),
                "masks": masks_u16,
            }
        )

    try:
        res = run_bass_kernel_spmd(nc, in_maps, core_ids=list(range(N_CORES)))
    except Exception:
        # transient NRT device wedges happen; one retry is usually enough
        res = run_bass_kernel_spmd(nc, in_maps, core_ids=list(range(N_CORES)))

    bv = b_attn[2 * C : 3 * C]
    const_bias = (bv @ W_proj + b_proj).astype(np.float32)  # [C]
    out = np.empty((B, T, C), np.float32)
    for b in range(B):
        out[b] = res.results[2 * b]["out"] + res.results[2 * b + 1]["out"] + const_bias
    return out


# revision 15
# speedup vs baseline: 1.2635x; 1.2635x over previous
"""Causal self-attention on 8 TRN2 NeuronCores.

Reference computation (B=4, T=2048, C=1024, H=16, D=64, fp32):
    qkv = x @ W_attn + b_attn ; split q,k,v ; per-head causal softmax(q k^T / 8) @ v
    y = heads @ W_proj + b_proj

Sharding: core c handles batch b = c//2 and head-half hh = c%2 (8 heads).
QKV weights are column-split and W_proj row-split per core, so each core
computes an independent partial projection; the host sums the two partials
per batch and adds the bias terms (b_proj and the folded-out v-bias
contribution b_v @ W_proj, which is constant because softmax rows sum to 1).
No collectives, no redundant FLOPs.

Per-core kernel layout:
  - qT, kT are feature-major [head*64, T]; v is token-major with a ones
    column per head ([T, 8 x (64 v | 1)]).
  - S^T tiles [k=128, q<=512] come from K=64 matmuls, two heads sharing the
    PE array via partition-offset row groups; exp reads both heads' PSUM
    banks in one ACT op; causal masking is a DVE multiply with a triangle
    mask on the diagonal 128-blocks only; above-diagonal work is skipped at
    128-col granularity (suffix trimming) for every q-quarter.
  - A@V runs "flipped": the exp tile is the stationary operand and the
    augmented v chunk [64 v | 1] the moving one, so each matmul's output is
    [128 q, 65] instead of [65, 512] - ~2x less PE streaming for the same
    math. Row 64 accumulates the softmax denominator. One DVE divide per
    (head, q-subtile) normalizes straight out of PSUM into a bf16 [q, d]
    tile, and a single DMA-transpose per (head-pair, quarter) rebuilds the
    feature-major yT used as the projection's stationary operand.
  - Emission order doubles as scheduler priority: QKV groups of the next
    t-quarter and projection groups of finished quarters are interleaved
    between attention head-pairs, so the PE always has ready work while
    the ACT engine grinds through exp.

Dtypes: matmul inputs in bf16 (halves DMA; fp32 PSUM accumulation keeps the
end-to-end error ~2.6e-3 against the 2e-2 gate).
"""

import numpy as np
import ml_dtypes

import concourse.bacc as bacc
import concourse.mybir as mybir
import concourse.tile as tile
from concourse.bass_utils import run_bass_kernel_spmd

F32 = mybir.dt.float32
BF16 = mybir.dt.bfloat16
FP8 = mybir.dt.float8e4
AF = mybir.ActivationFunctionType
ALU = mybir.AluOpType
DR = mybir.MatmulPerfMode.DoubleRow

N_CORES = 8
B, T, C = 4, 2048, 1024
H, D = 16, 64
CH = 512            # features per core (8 heads * 64)
NFO = 4             # head-pair chunks of 128 features
NTQ = 4             # t quarters of 512
NTC = 16            # t chunks of 128
SCALE = 0.125       # 1/sqrt(64)

_cached = {}


def _build_nc():
    nc = bacc.Bacc("TRN2", debug=False, num_devices=N_CORES)

    d_xT = nc.dram_tensor("xT", [C, T], BF16, kind="ExternalInput")
    d_wq = nc.dram_tensor("wq", [C, CH], BF16, kind="ExternalInput")
    d_wk = nc.dram_tensor("wk", [C, CH], BF16, kind="ExternalInput")
    d_wv = nc.dram_tensor("wv", [C, CH], BF16, kind="ExternalInput")
    d_bq = nc.dram_tensor("bq", [128, NFO], F32, kind="ExternalInput")
    d_bk = nc.dram_tensor("bk", [128, NFO], F32, kind="ExternalInput")
    d_wp = nc.dram_tensor("wp", [CH, C], BF16, kind="ExternalInput")
    d_masks = nc.dram_tensor("masks", [128, 128], BF16, kind="ExternalInput")
    d_out = nc.dram_tensor("out", [T, C], F32, kind="ExternalOutput")

    with tile.TileContext(nc) as tc, nc.allow_low_precision(
        reason="bf16 staging; accumulation stays fp32 in PSUM"
    ), (
        tc.tile_pool(name="persist", bufs=1)
    ) as persist, (
        tc.tile_pool(name="pW", bufs=1)
    ) as pW, (
        tc.tile_pool(name="pX", bufs=1)
    ) as pX, (
        tc.tile_pool(name="pO", bufs=4)
    ) as pO, (
        tc.tile_pool(name="p2e", bufs=6)
    ) as p2e, (
        tc.tile_pool(name="pY", bufs=2)
    ) as pY, (
        tc.tile_pool(name="psA", bufs=2, space="PSUM")
    ) as psA, (
        tc.tile_pool(name="psS", bufs=2, space="PSUM")
    ) as psS, (
        tc.tile_pool(name="psY", bufs=1, space="PSUM")
    ) as psY:
        # persistent on-chip tensors.
        # q/k live in fp8 DoubleRow layout: tile g in {0,1} holds head-pairs
        # fo = 2g, 2g+1; partition p = (fo%2)*64 + j*32 + dk and free slot
        # (i, t) hold feature d = 32*i + dk of head 2*fo+j. A DoubleRow
        # matmul contracts the (partition, slot) pair - 64 d-values on 32
        # partitions - at half the PE cost per output column.
        qdr = [persist.tile([128, 2, T], FP8, tag=f"qdr{g}", name=f"qdr{g}") for g in range(2)]
        kdr = [persist.tile([128, 2, T], FP8, tag=f"kdr{g}", name=f"kdr{g}") for g in range(2)]
        v = [persist.tile([128, 8, 65], BF16, tag=f"v{i}", name=f"v{i}") for i in range(NTC)]
        yT = [persist.tile([128, T], BF16, tag=f"yT{fo}", name=f"yT{fo}") for fo in range(NFO)]
        bq_sb = persist.tile([128, NFO], F32, tag="bq")
        bk_sb = persist.tile([128, NFO], F32, tag="bk")
        masks_sb = persist.tile([128, 128], BF16, tag="masks")
        wq_sb = pW.tile([128, 8, CH], BF16, tag="wq")
        wk_sb = pW.tile([128, 8, CH], BF16, tag="wk")
        wv_sb = pW.tile([128, 8, CH], BF16, tag="wv")
        wp_sb = pW.tile([128, 4, C], BF16, tag="wp")
        x_tiles = [pX.tile([128, 8, 512], BF16, tag=f"x{tq}", name=f"x{tq}") for tq in range(NTQ)]

        # input DMAs: first-needed tensors split chunk-size so the first
        # QKV matmuls can start as soon as possible; later tensors ride in
        # bigger transfers (per-DMA dispatch is ~650ns, keep the count low)
        def _x_piece(tq, c0, c1, t0=0, t1=512):
            nc.sync.dma_start(
                x_tiles[tq][:, c0:c1, t0:t1],
                d_xT.ap()[128 * c0 : 128 * c1, 512 * tq + t0 : 512 * tq + t1].rearrange(
                    "(c p) t -> p c t", p=128
                ),
            )

        def _w_piece(dst, src, c0, c1):
            nc.sync.dma_start(
                dst[:, c0:c1, :],
                src.ap()[128 * c0 : 128 * c1, :].rearrange("(c p) f -> p c f", p=128),
            )

        _x_piece(0, 0, 1)
        _w_piece(wq_sb, d_wq, 0, 1)
        nc.sync.dma_start(bq_sb[:], d_bq.ap())
        nc.sync.dma_start(bk_sb[:], d_bk.ap())
        _x_piece(0, 1, 2)
        _w_piece(wq_sb, d_wq, 1, 2)
        _x_piece(0, 2, 4)
        _w_piece(wq_sb, d_wq, 2, 4)
        _x_piece(0, 4, 8)
        _w_piece(wq_sb, d_wq, 4, 8)
        _w_piece(wk_sb, d_wk, 0, 4)
        _w_piece(wv_sb, d_wv, 0, 4)
        _w_piece(wk_sb, d_wk, 4, 8)
        _w_piece(wv_sb, d_wv, 4, 8)
        nc.sync.dma_start(masks_sb[:], d_masks.ap())
        for tq in range(1, NTQ):
            nc.sync.dma_start(
                x_tiles[tq][:],
                d_xT.ap()[:, 512 * tq : 512 * (tq + 1)].rearrange("(c p) t -> p c t", p=128),
            )
        nc.sync.dma_start(wp_sb[:], d_wp.ap().rearrange("(c p) f -> p c f", p=128))

        def emit_qk_group(bq_, w_sb, b_sb, dst, mi):
            # one full M=128 accumulation chain per (fp8 tile m, DoubleRow
            # slot i): the host pre-permutes W columns to [m, i, f, j, dk]
            # order, so the chain's 128 output partitions are exactly tile
            # m's partition layout for slot i (both head-pairs at once).
            m, i = divmod(mi, 2)
            ps = psA.tile([128, 512], F32, tag="psA", name="ps_qk")
            for ci in range(8):
                nc.tensor.matmul(
                    ps[:],
                    w_sb[:, ci, 256 * m + 128 * i : 256 * m + 128 * (i + 1)],
                    x_tiles[bq_][:, ci, :],
                    start=(ci == 0),
                    stop=(ci == 7),
                )
            nc.vector.tensor_scalar(
                dst[m][:, i, 512 * bq_ : 512 * (bq_ + 1)],
                ps[:],
                b_sb[:, mi : mi + 1],
                None,
                op0=ALU.add,
            )

        def emit_v_group(bq_, ts_):
            tci = 4 * bq_ + ts_
            ps = psA.tile([128, 512], F32, tag="psA", name="ps_v")
            for ci in range(8):
                nc.tensor.matmul(
                    ps[:],
                    x_tiles[bq_][:, ci, 128 * ts_ : 128 * (ts_ + 1)],
                    wv_sb[:, ci, :],
                    start=(ci == 0),
                    stop=(ci == 7),
                )
            nc.vector.memset(v[tci][:, :, 64:65], 1.0)
            nc.vector.tensor_copy(
                v[tci][:, :, 0:64],
                ps[:].rearrange("p (h d) -> p h d", h=8),
            )

        def emit_qkv_group(bq_, g):
            if g < 4:
                emit_qk_group(bq_, wq_sb, bq_sb, qdr, g)
            elif g < 8:
                emit_qk_group(bq_, wk_sb, bk_sb, kdr, g - 4)
            else:
                emit_v_group(bq_, g - 8)

        def emit_proj_tc(tci):
            o_sb = pO.tile([128, C], F32, tag="o", name="o_sb")
            for co in range(2):
                ps = psA.tile([128, 512], F32, tag="psA", name="ps_o")
                for fo in range(NFO):
                    nc.tensor.matmul(
                        ps[:],
                        yT[fo][:, 128 * tci : 128 * (tci + 1)],
                        wp_sb[:, fo, 512 * co : 512 * (co + 1)],
                        start=(fo == 0),
                        stop=(fo == 3),
                    )
                # both copies on DVE: ACT must stay free for exp, which is
                # the serial resource gating the attention pipeline
                nc.vector.tensor_copy(o_sb[:, 512 * co : 512 * (co + 1)], ps[:])
                nc.sync.dma_start(
                    d_out.ap()[128 * tci : 128 * (tci + 1), 512 * co : 512 * (co + 1)],
                    o_sb[:, 512 * co : 512 * (co + 1)],
                )

        def emit_attn(fo, b):
            hA, hB = 2 * fo, 2 * fo + 1
            q0 = 512 * b
            # A@V accumulators: per head, 4 q-subtiles of [128 q, 65] at
            # 128-col offsets inside one PSUM bank; col 64 is the softmax
            # denominator fed by the ones column of v.
            pyA = psY.tile([128, 512], F32, tag="pyA")
            pyB = psY.tile([128, 512], F32, tag="pyB")
            # zero the accumulator regions explicitly: matmul start=True
            # would lazily zero the whole 2KB zero-region (the bank), which
            # breaks interleaved per-subtile accumulation groups. The memset
            # overlaps every region, so it also orders all A@V matmuls after
            # it regardless of scheduler priority.
            nc.vector.memset(pyA[:].rearrange("p (s c) -> p s c", c=128)[:, :, 0:65], 0.0)
            nc.vector.memset(pyB[:].rearrange("p (s c) -> p s c", c=128)[:, :, 0:65], 0.0)
            # diagonal chunks first (kc = 4b+i), then full rows; chunk i only
            # reaches q-subtiles s >= i, everything above the diagonal is
            # skipped at 128-col granularity.
            order = [4 * b + i for i in range(4)] + list(range(4 * b))
            n_row = 4 * b  # full-row chunks
            for idx, kc in enumerate(order):
                i = kc - 4 * b  # >=0 for diagonal chunks
                qoff = 0 if i <= 0 else 128 * i
                pS = psS.tile([128, 1024], F32, tag="pS", name="pS")
                g = fo // 2
                for j, ooff in ((0, 0), (1, 512)):
                    pb = (fo % 2) * 64 + 32 * j
                    nc.tensor.matmul(
                        pS[:, ooff + qoff : ooff + 512],
                        kdr[g][pb : pb + 32, :, 128 * kc : 128 * (kc + 1)],
                        qdr[g][pb : pb + 32, :, q0 + qoff : q0 + 512],
                        perf_mode=DR,
                        tile_position=(pb, 0),
                    )
                eST = p2e.tile([128, 1024], BF16, tag="eST", name="eST")
                if qoff == 0:
                    nc.scalar.activation(eST[:], pS[:], AF.Exp, scale=SCALE)
                else:
                    # one ACT op over both heads' valid suffixes, skipping the
                    # [512, 512+qoff) hole via a strided AP
                    pS2 = pS[:].rearrange("p (two n) -> p two n", two=2)
                    eST2 = eST[:].rearrange("p (two n) -> p two n", two=2)
                    nc.scalar.activation(
                        eST2[:, :, qoff:512], pS2[:, :, qoff:512], AF.Exp, scale=SCALE
                    )
                if i >= 0:
                    # causal triangle mask on the diagonal 128-block
                    for off in (qoff, 512 + qoff):
                        nc.vector.tensor_tensor(
                            eST[:, off : off + 128],
                            eST[:, off : off + 128],
                            masks_sb[:],
                            op=ALU.mult,
                        )
                subs = range(i, 4) if i >= 0 else range(4)
                for s in subs:
                    nc.tensor.matmul(
                        pyA[:, 128 * s : 128 * s + 65],
                        eST[:, 128 * s : 128 * (s + 1)],
                        v[kc][:, hA, :],
                        start=False,
                        stop=False,
                        skip_group_check=True,
                    )
                    nc.tensor.matmul(
                        pyB[:, 128 * s : 128 * s + 65],
                        eST[:, 512 + 128 * s : 512 + 128 * (s + 1)],
                        v[kc][:, hB, :],
                        start=False,
                        stop=False,
                        skip_group_check=True,
                    )
            # normalize: reciprocal of the per-q denominators (col 64 of each
            # region), then one DVE multiply per (head, q-subtile) straight
            # out of PSUM into the bf16 [q, (s, d)] staging tile; a single
            # DMA-transpose rebuilds feature-major yT.
            yQ = pY.tile([128, 4, 128], BF16, tag="yQ", name="yQ")
            recA = pY.tile([128, 4], F32, tag="recA", name="recA")
            recB = pY.tile([128, 4], F32, tag="recB", name="recB")
            pyA4 = pyA[:].rearrange("p (s c) -> p s c", c=128)
            pyB4 = pyB[:].rearrange("p (s c) -> p s c", c=128)
            nc.vector.reciprocal(recA[:], pyA4[:, :, 64:65])
            nc.vector.reciprocal(recB[:], pyB4[:, :, 64:65])
            for s in range(4):
                nc.vector.tensor_scalar(
                    yQ[:, s, 0:64],
                    pyA[:, 128 * s : 128 * s + 64],
                    recA[:, s : s + 1],
                    None,
                    op0=ALU.mult,
                )
                nc.vector.tensor_scalar(
                    yQ[:, s, 64:128],
                    pyB[:, 128 * s : 128 * s + 64],
                    recB[:, s : s + 1],
                    None,
                    op0=ALU.mult,
                )
            nc.sync.dma_start_transpose(
                yT[fo][:, q0 : q0 + 512].rearrange("p (s q) -> p s q", s=4),
                yQ[:],
            )

        # Emission order doubles as scheduler priority: between attention
        # head-pairs (whose inner loop is ACT-bound) we emit PE-dense filler:
        # the NEXT quarter's QKV groups, or projection groups of a finished
        # quarter, so the PE always has ready work.
        for g in range(12):
            emit_qkv_group(0, g)
        for b in range(NTQ):
            for fo in range(NFO):
                emit_attn(fo, b)
                if b < NTQ - 1:
                    for g in range(3 * fo, 3 * fo + 3):
                        emit_qkv_group(b + 1, g)
                if b == 2:
                    emit_proj_tc(fo)          # proj quarter 0
                elif b == 3:
                    # b=3 has no QKV filler left; give each head-pair
                    # boundary two projection units (quarters 1 and 2)
                    emit_proj_tc(4 + fo)
                    emit_proj_tc(8 + fo)
        for tci in range(12, 16):
            emit_proj_tc(tci)

    nc.compile()
    return nc


def _get_nc():
    if "nc" not in _cached:
        _cached["nc"] = _build_nc()
    return _cached["nc"]


def kernel(x, W_attn, b_attn, W_proj, b_proj):
    x = np.asarray(x, np.float32)
    W_attn = np.asarray(W_attn, np.float32)
    b_attn = np.asarray(b_attn, np.float32)
    W_proj = np.asarray(W_proj, np.float32)
    b_proj = np.asarray(b_proj, np.float32)

    nc = _get_nc()
    p = np.arange(128)[:, None]
    j = np.arange(128)[None, :]
    tri = (j >= p).astype(np.float32)          # [128,128] valid iff j >= p
    masks = tri.astype(ml_dtypes.bfloat16)     # [128, 128]
    masks_u16 = masks.view(np.uint16)

    # q/k weight-column permutation for the fp8 DoubleRow layout: new column
    # m*256 + i*128 + f*64 + j*32 + dk holds feature (4m+2f+j)*64 + 32i + dk,
    # so each M=128 QKV chain (m, i) lands directly on fp8 tile m's partition
    # layout for DoubleRow slot i.
    m_, i_, f_, j_, dk_ = np.meshgrid(
        np.arange(2), np.arange(2), np.arange(2), np.arange(2), np.arange(32),
        indexing="ij",
    )
    perm = ((4 * m_ + 2 * f_ + j_) * 64 + 32 * i_ + dk_).reshape(-1)   # [512]

    def dr_bias(bvec):
        # [128, 4]: partition f*64 + j*32 + dk, column m*2 + i
        arr = np.zeros((128, 4), np.float32)
        arr[(f_ * 64 + j_ * 32 + dk_).reshape(-1),
            (m_ * 2 + i_).reshape(-1)] = bvec[perm]
        return arr

    in_maps = []
    for c in range(N_CORES):
        b, hh = divmod(c, 2)
        sl = slice(CH * hh, CH * (hh + 1))
        in_maps.append(
            {
                "xT": np.ascontiguousarray(x[b].T).astype(ml_dtypes.bfloat16).view(np.uint16),
                "wq": np.ascontiguousarray(W_attn[:, 0:C][:, sl][:, perm]).astype(ml_dtypes.bfloat16).view(np.uint16),
                "wk": np.ascontiguousarray(W_attn[:, C : 2 * C][:, sl][:, perm]).astype(ml_dtypes.bfloat16).view(np.uint16),
                "wv": np.ascontiguousarray(W_attn[:, 2 * C : 3 * C][:, sl]).astype(ml_dtypes.bfloat16).view(np.uint16),
                "bq": dr_bias(b_attn[0:C][sl]),
                "bk": dr_bias(b_attn[C : 2 * C][sl]),
                "wp": np.ascontiguousarray(
                    W_proj[sl, :].astype(ml_dtypes.bfloat16)
                ).view(np.uint16),
                "masks": masks_u16,
            }
        )

    try:
        res = run_bass_kernel_spmd(nc, in_maps, core_ids=list(range(N_CORES)))
    except Exception:
        # transient NRT device wedges happen; one retry is usually enough
        res = run_bass_kernel_spmd(nc, in_maps, core_ids=list(range(N_CORES)))

    bv = b_attn[2 * C : 3 * C]
    const_bias = (bv @ W_proj + b_proj).astype(np.float32)  # [C]
    out = np.empty((B, T, C), np.float32)
    for b in range(B):
        out[b] = res.results[2 * b]["out"] + res.results[2 * b + 1]["out"] + const_bias
    return out


# revision 20
# speedup vs baseline: 1.3140x; 1.0400x over previous
"""Causal self-attention on 8 TRN2 NeuronCores.

Reference computation (B=4, T=2048, C=1024, H=16, D=64, fp32):
    qkv = x @ W_attn + b_attn ; split q,k,v ; per-head causal softmax(q k^T / 8) @ v
    y = heads @ W_proj + b_proj

Sharding: core c handles batch b = c//2 and head-half hh = c%2 (8 heads).
QKV weights are column-split and W_proj row-split per core, so each core
computes an independent partial projection; the host sums the two partials
per batch and adds the bias terms (b_proj and the folded-out v-bias
contribution b_v @ W_proj, which is constant because softmax rows sum to 1).
No collectives, no redundant FLOPs.

Per-core kernel layout:
  - qT, kT are feature-major [head*64, T]; v is token-major with a ones
    column per head ([T, 8 x (64 v | 1)]).
  - S^T tiles [k=128, q<=512] come from K=64 matmuls, two heads sharing the
    PE array via partition-offset row groups; exp reads both heads' PSUM
    banks in one ACT op; causal masking is a DVE multiply with a triangle
    mask on the diagonal 128-blocks only; above-diagonal work is skipped at
    128-col granularity (suffix trimming) for every q-quarter.
  - A@V runs "flipped": the exp tile is the stationary operand and the
    augmented v chunk [64 v | 1] the moving one, so each matmul's output is
    [128 q, 65] instead of [65, 512] - ~2x less PE streaming for the same
    math. Row 64 accumulates the softmax denominator. One DVE divide per
    (head, q-subtile) normalizes straight out of PSUM into a bf16 [q, d]
    tile, and a single DMA-transpose per (head-pair, quarter) rebuilds the
    feature-major yT used as the projection's stationary operand.
  - Emission order doubles as scheduler priority: QKV groups of the next
    t-quarter and projection groups of finished quarters are interleaved
    between attention head-pairs, so the PE always has ready work while
    the ACT engine grinds through exp.

Dtypes: matmul inputs in bf16 (halves DMA; fp32 PSUM accumulation keeps the
end-to-end error ~2.6e-3 against the 2e-2 gate).
"""

import numpy as np
import ml_dtypes

import concourse.bacc as bacc
import concourse.mybir as mybir
import concourse.tile as tile
from concourse.bass_utils import run_bass_kernel_spmd

F32 = mybir.dt.float32
BF16 = mybir.dt.bfloat16
FP8 = mybir.dt.float8e4
AF = mybir.ActivationFunctionType
ALU = mybir.AluOpType
DR = mybir.MatmulPerfMode.DoubleRow

N_CORES = 8
B, T, C = 4, 2048, 1024
H, D = 16, 64
CH = 512            # features per core (8 heads * 64)
NFO = 4             # head-pair chunks of 128 features
NTQ = 4             # t quarters of 512
NTC = 16            # t chunks of 128
SCALE = 0.125       # 1/sqrt(64)

_cached = {}


def _build_nc():
    nc = bacc.Bacc("TRN2", debug=False, num_devices=N_CORES)

    d_xT = nc.dram_tensor("xT", [C, T], BF16, kind="ExternalInput")
    d_wq = nc.dram_tensor("wq", [C, CH], BF16, kind="ExternalInput")
    d_wk = nc.dram_tensor("wk", [C, CH], BF16, kind="ExternalInput")
    d_wv = nc.dram_tensor("wv", [C, CH], BF16, kind="ExternalInput")
    d_bq = nc.dram_tensor("bq", [128, NFO], F32, kind="ExternalInput")
    d_bk = nc.dram_tensor("bk", [128, NFO], F32, kind="ExternalInput")
    d_wp = nc.dram_tensor("wp", [CH, C], BF16, kind="ExternalInput")
    d_masks = nc.dram_tensor("masks", [128, 128], BF16, kind="ExternalInput")
    d_out = nc.dram_tensor("out", [T, C], F32, kind="ExternalOutput")

    with tile.TileContext(nc) as tc, nc.allow_low_precision(
        reason="bf16 staging; accumulation stays fp32 in PSUM"
    ), (
        tc.tile_pool(name="persist", bufs=1)
    ) as persist, (
        tc.tile_pool(name="pW", bufs=1)
    ) as pW, (
        tc.tile_pool(name="pX", bufs=1)
    ) as pX, (
        tc.tile_pool(name="pO", bufs=4)
    ) as pO, (
        tc.tile_pool(name="p2e", bufs=6)
    ) as p2e, (
        tc.tile_pool(name="pY", bufs=2)
    ) as pY, (
        tc.tile_pool(name="psA", bufs=2, space="PSUM")
    ) as psA, (
        tc.tile_pool(name="psS", bufs=2, space="PSUM")
    ) as psS, (
        tc.tile_pool(name="psY", bufs=1, space="PSUM")
    ) as psY:
        # persistent on-chip tensors.
        # q/k live in fp8 DoubleRow layout: tile g in {0,1} holds head-pairs
        # fo = 2g, 2g+1; partition p = (fo%2)*64 + j*32 + dk and free slot
        # (i, t) hold feature d = 32*i + dk of head 2*fo+j. A DoubleRow
        # matmul contracts the (partition, slot) pair - 64 d-values on 32
        # partitions - at half the PE cost per output column.
        qdr = [persist.tile([128, 2, T], FP8, tag=f"qdr{g}", name=f"qdr{g}") for g in range(2)]
        kdr = [persist.tile([128, 2, T], FP8, tag=f"kdr{g}", name=f"kdr{g}") for g in range(2)]
        v = [persist.tile([128, 8, 65], BF16, tag=f"v{i}", name=f"v{i}") for i in range(NTC)]
        yT = [persist.tile([128, T], BF16, tag=f"yT{fo}", name=f"yT{fo}") for fo in range(NFO)]
        bq_sb = persist.tile([128, NFO], F32, tag="bq")
        bk_sb = persist.tile([128, NFO], F32, tag="bk")
        masks_sb = persist.tile([128, 128], BF16, tag="masks")
        wq_sb = pW.tile([128, 8, CH], BF16, tag="wq")
        wk_sb = pW.tile([128, 8, CH], BF16, tag="wk")
        wv_sb = pW.tile([128, 8, CH], BF16, tag="wv")
        wp_sb = pW.tile([128, 4, C], BF16, tag="wp")
        x_tiles = [pX.tile([128, 8, 512], BF16, tag=f"x{tq}", name=f"x{tq}") for tq in range(NTQ)]

        # input DMAs: first-needed tensors split chunk-size so the first
        # QKV matmuls can start as soon as possible; later tensors ride in
        # bigger transfers (per-DMA dispatch is ~650ns, keep the count low)
        def _x_piece(tq, c0, c1, t0=0, t1=512):
            nc.sync.dma_start(
                x_tiles[tq][:, c0:c1, t0:t1],
                d_xT.ap()[128 * c0 : 128 * c1, 512 * tq + t0 : 512 * tq + t1].rearrange(
                    "(c p) t -> p c t", p=128
                ),
            )

        def _w_piece(dst, src, c0, c1):
            nc.sync.dma_start(
                dst[:, c0:c1, :],
                src.ap()[128 * c0 : 128 * c1, :].rearrange("(c p) f -> p c f", p=128),
            )

        _x_piece(0, 0, 1)
        _w_piece(wq_sb, d_wq, 0, 1)
        nc.sync.dma_start(bq_sb[:], d_bq.ap())
        nc.sync.dma_start(bk_sb[:], d_bk.ap())
        _x_piece(0, 1, 3)
        _w_piece(wq_sb, d_wq, 1, 3)
        _x_piece(0, 3, 8)
        _w_piece(wq_sb, d_wq, 3, 8)
        _w_piece(wk_sb, d_wk, 0, 4)
        _w_piece(wv_sb, d_wv, 0, 4)
        _w_piece(wk_sb, d_wk, 4, 8)
        _w_piece(wv_sb, d_wv, 4, 8)
        nc.sync.dma_start(masks_sb[:], d_masks.ap())
        for tq in range(1, NTQ):
            nc.sync.dma_start(
                x_tiles[tq][:],
                d_xT.ap()[:, 512 * tq : 512 * (tq + 1)].rearrange("(c p) t -> p c t", p=128),
            )
        nc.sync.dma_start(wp_sb[:], d_wp.ap().rearrange("(c p) f -> p c f", p=128))

        def emit_qk_group(bq_, w_sb, b_sb, dst, mi):
            # one full M=128 accumulation chain per (fp8 tile m, DoubleRow
            # slot i): the host pre-permutes W columns to [m, i, f, j, dk]
            # order, so the chain's 128 output partitions are exactly tile
            # m's partition layout for slot i (both head-pairs at once).
            m, i = divmod(mi, 2)
            ps = psA.tile([128, 512], F32, tag="psA", name="ps_qk")
            for ci in range(8):
                nc.tensor.matmul(
                    ps[:],
                    w_sb[:, ci, 256 * m + 128 * i : 256 * m + 128 * (i + 1)],
                    x_tiles[bq_][:, ci, :],
                    start=(ci == 0),
                    stop=(ci == 7),
                )
            nc.vector.tensor_scalar(
                dst[m][:, i, 512 * bq_ : 512 * (bq_ + 1)],
                ps[:],
                b_sb[:, mi : mi + 1],
                None,
                op0=ALU.add,
            )

        def emit_v_group(bq_, ts_):
            tci = 4 * bq_ + ts_
            ps = psA.tile([128, 512], F32, tag="psA", name="ps_v")
            for ci in range(8):
                nc.tensor.matmul(
                    ps[:],
                    x_tiles[bq_][:, ci, 128 * ts_ : 128 * (ts_ + 1)],
                    wv_sb[:, ci, :],
                    start=(ci == 0),
                    stop=(ci == 7),
                )
            nc.vector.memset(v[tci][:, :, 64:65], 1.0)
            nc.vector.tensor_copy(
                v[tci][:, :, 0:64],
                ps[:].rearrange("p (h d) -> p h d", h=8),
            )

        def emit_qkv_group(bq_, g):
            if g < 4:
                emit_qk_group(bq_, wq_sb, bq_sb, qdr, g)
            elif g < 8:
                emit_qk_group(bq_, wk_sb, bk_sb, kdr, g - 4)
            else:
                emit_v_group(bq_, g - 8)

        def emit_proj_tc(tci, pool=None):
            o_sb = pO.tile([128, C], F32, tag="o", name="o_sb")
            for co in range(2):
                if pool is None or pool is psA:
                    ps = psA.tile([128, 512], F32, tag="psA", name="ps_o")
                else:
                    ps = pool.tile([128, 512], F32, tag="pyA" if co == 0 else "pyB", name="ps_o")
                for fo in range(NFO):
                    nc.tensor.matmul(
                        ps[:],
                        yT[fo][:, 128 * tci : 128 * (tci + 1)],
                        wp_sb[:, fo, 512 * co : 512 * (co + 1)],
                        start=(fo == 0),
                        stop=(fo == 3),
                    )
                # both copies on DVE: ACT must stay free for exp, which is
                # the serial resource gating the attention pipeline
                nc.vector.tensor_copy(o_sb[:, 512 * co : 512 * (co + 1)], ps[:])
                nc.sync.dma_start(
                    d_out.ap()[128 * tci : 128 * (tci + 1), 512 * co : 512 * (co + 1)],
                    o_sb[:, 512 * co : 512 * (co + 1)],
                )

        def emit_attn(fo, b):
            hA, hB = 2 * fo, 2 * fo + 1
            q0 = 512 * b
            # A@V accumulators: per head, 4 q-subtiles of [128 q, 65] at
            # 128-col offsets inside one PSUM bank; col 64 is the softmax
            # denominator fed by the ones column of v.
            pyA = psY.tile([128, 512], F32, tag="pyA")
            pyB = psY.tile([128, 512], F32, tag="pyB")
            # zero the accumulator regions explicitly: matmul start=True
            # would lazily zero the whole 2KB zero-region (the bank), which
            # breaks interleaved per-subtile accumulation groups. The memset
            # overlaps every region, so it also orders all A@V matmuls after
            # it regardless of scheduler priority.
            nc.vector.memset(pyA[:].rearrange("p (s c) -> p s c", c=128)[:, :, 0:65], 0.0)
            nc.vector.memset(pyB[:].rearrange("p (s c) -> p s c", c=128)[:, :, 0:65], 0.0)
            # diagonal chunks first (kc = 4b+i), then full rows; chunk i only
            # reaches q-subtiles s >= i, everything above the diagonal is
            # skipped at 128-col granularity.
            order = [4 * b + i for i in range(4)] + list(range(4 * b))
            n_row = 4 * b  # full-row chunks
            for idx, kc in enumerate(order):
                i = kc - 4 * b  # >=0 for diagonal chunks
                qoff = 0 if i <= 0 else 128 * i
                pS = psS.tile([128, 1024], F32, tag="pS", name="pS")
                g = fo // 2
                for j, ooff in ((0, 0), (1, 512)):
                    pb = (fo % 2) * 64 + 32 * j
                    nc.tensor.matmul(
                        pS[:, ooff + qoff : ooff + 512],
                        kdr[g][pb : pb + 32, :, 128 * kc : 128 * (kc + 1)],
                        qdr[g][pb : pb + 32, :, q0 + qoff : q0 + 512],
                        perf_mode=DR,
                        tile_position=(pb, 0),
                    )
                eST = p2e.tile([128, 1024], BF16, tag="eST", name="eST")
                if qoff == 0:
                    nc.scalar.activation(eST[:], pS[:], AF.Exp, scale=SCALE)
                else:
                    # one ACT op over both heads' valid suffixes, skipping the
                    # [512, 512+qoff) hole via a strided AP
                    pS2 = pS[:].rearrange("p (two n) -> p two n", two=2)
                    eST2 = eST[:].rearrange("p (two n) -> p two n", two=2)
                    nc.scalar.activation(
                        eST2[:, :, qoff:512], pS2[:, :, qoff:512], AF.Exp, scale=SCALE
                    )
                if i >= 0:
                    # causal triangle mask on the diagonal 128-block
                    for off in (qoff, 512 + qoff):
                        nc.vector.tensor_tensor(
                            eST[:, off : off + 128],
                            eST[:, off : off + 128],
                            masks_sb[:],
                            op=ALU.mult,
                        )
                subs = range(i, 4) if i >= 0 else range(4)
                for s in subs:
                    nc.tensor.matmul(
                        pyA[:, 128 * s : 128 * s + 65],
                        eST[:, 128 * s : 128 * (s + 1)],
                        v[kc][:, hA, :],
                        start=False,
                        stop=False,
                        skip_group_check=True,
                    )
                    nc.tensor.matmul(
                        pyB[:, 128 * s : 128 * s + 65],
                        eST[:, 512 + 128 * s : 512 + 128 * (s + 1)],
                        v[kc][:, hB, :],
                        start=False,
                        stop=False,
                        skip_group_check=True,
                    )
            # normalize: reciprocal of the per-q denominators (col 64 of each
            # region), then one DVE multiply per (head, q-subtile) straight
            # out of PSUM into the bf16 [q, (s, d)] staging tile; a single
            # DMA-transpose rebuilds feature-major yT.
            yQ = pY.tile([128, 4, 128], BF16, tag="yQ", name="yQ")
            recA = pY.tile([128, 4], F32, tag="recA", name="recA")
            recB = pY.tile([128, 4], F32, tag="recB", name="recB")
            pyA4 = pyA[:].rearrange("p (s c) -> p s c", c=128)
            pyB4 = pyB[:].rearrange("p (s c) -> p s c", c=128)
            nc.vector.reciprocal(recA[:], pyA4[:, :, 64:65])
            nc.vector.reciprocal(recB[:], pyB4[:, :, 64:65])
            for s in range(4):
                nc.vector.tensor_scalar(
                    yQ[:, s, 0:64],
                    pyA[:, 128 * s : 128 * s + 64],
                    recA[:, s : s + 1],
                    None,
                    op0=ALU.mult,
                )
                nc.vector.tensor_scalar(
                    yQ[:, s, 64:128],
                    pyB[:, 128 * s : 128 * s + 64],
                    recB[:, s : s + 1],
                    None,
                    op0=ALU.mult,
                )
            nc.sync.dma_start_transpose(
                yT[fo][:, q0 : q0 + 512].rearrange("p (s q) -> p s q", s=4),
                yQ[:],
            )

        # Emission order doubles as scheduler priority: between attention
        # head-pairs (whose inner loop is ACT-bound) we emit PE-dense filler:
        # the NEXT quarter's QKV groups, or projection groups of a finished
        # quarter, so the PE always has ready work. Group order within a
        # quarter is chosen so the tensors the next quarter's FIRST attention
        # blocks depend on (m=0 q/k chains, then v) are produced first - the
        # ACT exp stream, the serial resource, must never wait on them.
        # filler[(b, fo)]: QKV groups (quarter, g) and proj units emitted
        # after attention block (fo, b). Invariant: a quarter's m0 q/k
        # chains (g 0,1,4,5) and all v groups (8-11) are emitted before its
        # first attention block; only the m1 chains (2,3,6,7) trail into the
        # quarter itself (first used by its fo=2 block). Movable proj work
        # is held back for the b>=2 blocks, whose ACT-gated stream would
        # otherwise leave the PE idle.
        filler = {(0, 0): [(0, 2), (0, 3)], (0, 1): [(0, 6), (0, 7)]}
        for q in range(1, NTQ):
            filler[(q - 1, 1)] += [(q, 0), (q, 1)]
            filler[(q - 1, 2)] = [(q, 4), (q, 5), (q, 8)]
            filler[(q - 1, 3)] = [(q, 9), (q, 10), (q, 11)]
            filler[(q, 0)] = [(q, 2), (q, 3)]
            filler[(q, 1)] = [(q, 6), (q, 7)]
        proj_units = {(2, 0): [0], (2, 1): [1], (2, 2): [2], (2, 3): [3],
                      (3, 0): [4, 5], (3, 1): [6, 7], (3, 2): [8, 9],
                      (3, 3): [10, 11]}

        for g in [0, 1, 4, 5, 8, 9, 10, 11]:
            emit_qkv_group(0, g)
        for b in range(NTQ):
            for fo in range(NFO):
                emit_attn(fo, b)
                for q, g in filler.get((b, fo), []):
                    emit_qkv_group(q, g)
                for u in proj_units.get((b, fo), []):
                    emit_proj_tc(u)
        for tci in range(12, 16):
            # the last two units draw their PSUM from the attention
            # accumulator banks (free once the final normalize has read
            # them), so all four tail units run concurrently
            emit_proj_tc(tci, pool=psY if tci >= 14 else psA)

    nc.compile()
    return nc


def _get_nc():
    if "nc" not in _cached:
        _cached["nc"] = _build_nc()
    return _cached["nc"]


def kernel(x, W_attn, b_attn, W_proj, b_proj):
    x = np.asarray(x, np.float32)
    W_attn = np.asarray(W_attn, np.float32)
    b_attn = np.asarray(b_attn, np.float32)
    W_proj = np.asarray(W_proj, np.float32)
    b_proj = np.asarray(b_proj, np.float32)

    nc = _get_nc()
    p = np.arange(128)[:, None]
    j = np.arange(128)[None, :]
    tri = (j >= p).astype(np.float32)          # [128,128] valid iff j >= p
    masks = tri.astype(ml_dtypes.bfloat16)     # [128, 128]
    masks_u16 = masks.view(np.uint16)

    # q/k weight-column permutation for the fp8 DoubleRow layout: new column
    # m*256 + i*128 + f*64 + j*32 + dk holds feature (4m+2f+j)*64 + 32i + dk,
    # so each M=128 QKV chain (m, i) lands directly on fp8 tile m's partition
    # layout for DoubleRow slot i.
    m_, i_, f_, j_, dk_ = np.meshgrid(
        np.arange(2), np.arange(2), np.arange(2), np.arange(2), np.arange(32),
        indexing="ij",
    )
    perm = ((4 * m_ + 2 * f_ + j_) * 64 + 32 * i_ + dk_).reshape(-1)   # [512]

    def dr_bias(bvec):
        # [128, 4]: partition f*64 + j*32 + dk, column m*2 + i
        arr = np.zeros((128, 4), np.float32)
        arr[(f_ * 64 + j_ * 32 + dk_).reshape(-1),
            (m_ * 2 + i_).reshape(-1)] = bvec[perm]
        return arr

    in_maps = []
    for c in range(N_CORES):
        b, hh = divmod(c, 2)
        sl = slice(CH * hh, CH * (hh + 1))
        in_maps.append(
            {
                "xT": np.ascontiguousarray(x[b].T).astype(ml_dtypes.bfloat16).view(np.uint16),
                "wq": np.ascontiguousarray(W_attn[:, 0:C][:, sl][:, perm]).astype(ml_dtypes.bfloat16).view(np.uint16),
                "wk": np.ascontiguousarray(W_attn[:, C : 2 * C][:, sl][:, perm]).astype(ml_dtypes.bfloat16).view(np.uint16),
                "wv": np.ascontiguousarray(W_attn[:, 2 * C : 3 * C][:, sl]).astype(ml_dtypes.bfloat16).view(np.uint16),
                "bq": dr_bias(b_attn[0:C][sl]),
                "bk": dr_bias(b_attn[C : 2 * C][sl]),
                "wp": np.ascontiguousarray(
                    W_proj[sl, :].astype(ml_dtypes.bfloat16)
                ).view(np.uint16),
                "masks": masks_u16,
            }
        )

    try:
        res = run_bass_kernel_spmd(nc, in_maps, core_ids=list(range(N_CORES)))
    except Exception:
        # transient NRT device wedges happen; one retry is usually enough
        res = run_bass_kernel_spmd(nc, in_maps, core_ids=list(range(N_CORES)))

    bv = b_attn[2 * C : 3 * C]
    const_bias = (bv @ W_proj + b_proj).astype(np.float32)  # [C]
    out = np.empty((B, T, C), np.float32)
    for b in range(B):
        out[b] = res.results[2 * b]["out"] + res.results[2 * b + 1]["out"] + const_bias
    return out
